# revision 45
# baseline (speedup 1.0000x reference)
"""3-layer GAT on 8 TRN2 NeuronCores.

Sharding: nodes are globally degree-sorted and dealt round-robin across the
8 cores (rank k -> core k%8, local row k//8), so every core's group g (128
nodes = one SBUF partition-tile) holds nodes of near-identical degree and
the cross-core shared gather plan has minimal slot padding. All per-core
storage is in this local row order, which makes every producer/consumer DMA
contiguous: no indirect scatters, no scatter-row table. The host un-permutes
(and dequantizes) the final output.

Per layer: dense GEMM over own rows -> [h | asrc | adst] rows (f16, 384/128
elem row stride - dma_gather requires 256B-multiple strides) -> AllGather
into a replicated table -> per dst-group one low + one high dma_gather of
the in-edge source rows (int16 indices reach 32768 rows; the two windows
[0,32768) and [TROWS-32768, TROWS) overlap, and the host assigns edges in
the overlap to equalize per-group low/high slot maxima), unnormalized
softmax (exp into the tail of the f16 payload tile, one fused tensor_reduce
over edge slots for numerator+denominator), normalize + bias per group.
Dense layers consume the previous layer's output with dma_start_transpose
(XBAR) loads - no PE transposes. Leaky-relu is one Prelu activation (alpha
AP, vector-engine dynamic slices only - scalar-engine bias APs ignore
For_i-dynamic offsets). The last layer emits int8 values with a per-node
f16 scale packed into trailing bytes; the host dequantizes.

The wall clock is dominated by fixed per-call costs of the axon PJRT path
(fresh-jit lowering scaling with BIR size, ~95ms execute dispatch) and the
~35-45MB/s tunnel, so the design minimizes instruction count (For_i
hardware loops for the dense layers and for degree-bucketed aggregation
groups, DP-optimized unroll/bucket split) and bytes on the wire (one
byte-packed input param: f16 weights+biases | int16 indices | int8 x;
one packed int8+scale output).
"""
import numpy as np

from concourse import bacc, mybir, tile
from concourse.bass import ds
from concourse.bass_utils import run_bass_kernel_spmd

f32 = mybir.dt.float32
f16 = mybir.dt.float16
i8 = mybir.dt.int8
i16 = mybir.dt.int16
Alu = mybir.AluOpType
Act = mybir.ActivationFunctionType
AX = mybir.AxisListType

CFG = dict(
    N=50000, IN=128, HID=64, OUT=64, NH=4, R=8,
    VSPLIT=32768,
)


def make_cfg(**over):
    cfg = dict(CFG)
    cfg.update(over)
    N, R = cfg["N"], cfg["R"]
    assert N % R == 0
    cfg["SHARD"] = N // R                       # 6250
    cfg["LPAD"] = ((cfg["SHARD"] + 127) // 128) * 128   # 6272
    cfg["NG"] = cfg["LPAD"] // 128              # 49
    cfg["TROWS"] = R * cfg["LPAD"]              # 50176
    cfg["F"] = cfg["NH"] * cfg["HID"]           # 256
    cfg["W12"] = cfg["F"] + 2 * cfg["NH"]       # 264 natural row width L1/L2
    cfg["W3"] = cfg["OUT"] + 2                  # 66 natural row width L3
    cfg["GMAX"] = 384                           # gather elem_size L1/L2 (256B mult)
    cfg["GMAX3"] = 128                          # gather elem_size L3
    # low gather window [0, VSPLIT); high window [HBASE, TROWS). The two
    # overlap so the host can balance each node's low/high split.
    cfg["HBASE"] = max(0, cfg["TROWS"] - cfg["VSPLIT"])
    assert cfg["TROWS"] - cfg["HBASE"] <= cfg["VSPLIT"]
    # pad rows (asrc=-60000) for unused gather slots
    cfg["PAD_L"] = cfg["SHARD"]                 # core0 pad row, < VSPLIT
    pad_h = None
    for r in range(R):
        cand = r * cfg["LPAD"] + cfg["SHARD"]
        if cand >= cfg["HBASE"]:
            pad_h = cand
            break
    cfg["PAD_H"] = pad_h
    assert pad_h is not None and pad_h - cfg["HBASE"] < 32768
    return cfg


def _wrap_idx(idx_flat):
    """dma_gather compact int16 index layout: ordinal i at [i%16, i//16]."""
    n = len(idx_flat)
    assert n % 16 == 0
    return np.asarray(idx_flat, np.int16).reshape(n // 16, 16).T


def prep_host(x, edge_index, cfg):
    """Degree-balanced sharding. Returns (per_core dicts, plan, nix, dmax,
    xscale, byd) where plan[g] = (jl, jh, off_l, off_h) shared by all cores
    and byd[k] is the node with global degree rank k."""
    N, R, SHARD, LPAD, NG = cfg["N"], cfg["R"], cfg["SHARD"], cfg["LPAD"], cfg["NG"]
    VS, HB = cfg["VSPLIT"], cfg["HBASE"]
    src = np.concatenate([np.asarray(edge_index[0]), np.arange(N)]).astype(np.int64)
    dst = np.concatenate([np.asarray(edge_index[1]), np.arange(N)]).astype(np.int64)

    deg = np.bincount(dst, minlength=N)
    byd = np.argsort(-deg, kind="stable")       # rank -> node
    rank = np.empty(N, np.int64)
    rank[byd] = np.arange(N)
    core_of = rank % R
    row_of = rank // R                          # local row in [0, SHARD)
    trow_of = core_of * LPAD + row_of           # global table row

    tsrc = trow_of[src]
    dcore = core_of[dst]
    drow = row_of[dst]

    # sort edges by (dst node, src row) so each node's slice is value-sorted:
    # must-low [0,HB) | flexible [HB,VS) | must-high [VS,TROWS)
    node_key_all = dcore * SHARD + drow
    order = np.lexsort((tsrc, node_key_all))
    tsrc_s = tsrc[order]
    node_key = node_key_all[order]
    bounds = np.searchsorted(node_key, np.arange(R * SHARD + 1))

    # per-node (ml, ml+fx) counts; per (core,group) choose the low cap L_g
    # minimizing L + max_high across all cores
    srcs = []            # [core][local row] -> sorted src rows
    nml = np.zeros((R, SHARD), np.int64)
    nmf = np.zeros((R, SHARD), np.int64)
    ndeg = np.zeros((R, SHARD), np.int64)
    for c in range(R):
        per = []
        for lr in range(SHARD):
            s = tsrc_s[bounds[c * SHARD + lr]:bounds[c * SHARD + lr + 1]]
            per.append(s)
            nml[c, lr] = np.searchsorted(s, HB)
            nmf[c, lr] = np.searchsorted(s, VS)
            ndeg[c, lr] = len(s)
        srcs.append(per)

    # Per-group curves for the bucket DP: jh_g(L) = max_n(deg - min(mf, L)),
    # valid for L >= mlmax_g. Bucket [a,b) shares (jl,jh) = argmin_L of
    # (L + max_g jh_g(L)); slot padding trades against For_i loop overhead.
    LMAX = int(nmf.max()) + 1
    Ls = np.arange(LMAX + 1)
    jh_curve = np.zeros((NG, LMAX + 1), np.int64)
    mlmax = np.zeros(NG, np.int64)
    for g in range(NG):
        sl = slice(g * 128, min((g + 1) * 128, SHARD))
        ml = nml[:, sl].reshape(-1)
        mf = nmf[:, sl].reshape(-1)
        dg = ndeg[:, sl].reshape(-1)
        jh_curve[g] = (dg[None, :] - np.minimum(mf[None, :], Ls[:, None])).max(1)
        mlmax[g] = ml.max()

    def bucket_wh(a, b):
        lo = int(mlmax[a:b].max())
        jh = jh_curve[a:b].max(0)
        tot = np.maximum(Ls, lo) + jh
        L = int(np.argmin(tot[lo:]) + lo)
        return int(L), int(jh[L])

    # DP over contiguous splits: unrolled group ~4.1ms; For_i bucket
    # ~23.6ms fixed; each padded slot ~60us of idx H2D.
    UNROLL_US = 4056.0
    BUCKET_US = 23556.0
    SLOT_US = 60.0
    wmin = np.array([sum(bucket_wh(g, g + 1)) for g in range(NG)], np.float64)
    INF = float("inf")
    dp = np.full(NG + 1, INF)
    dp[NG] = 0.0
    choice = [None] * NG
    import os
    force_u = int(os.environ.get("GAT_FORCE_UNROLL", "0"))
    for a in range(NG - 1, -1, -1):
        dp[a] = UNROLL_US + dp[a + 1]
        choice[a] = ("u", a + 1)
        if a < force_u:
            continue
        for b in range(a + 2, NG + 1):
            jl, jh = bucket_wh(a, b)
            pad = ((jl + jh) * (b - a) - wmin[a:b].sum()) * SLOT_US
            c = BUCKET_US + pad + dp[b]
            if c < dp[a]:
                dp[a] = c
                choice[a] = ("b", b)
        # cap inner loop cost: O(NG^2) is fine at NG=49

    # emissions: ("u", g, jl, jh, off_l, off_h) or
    #            ("b", a, b, jl, jh, off_l, off_h)
    plan = []
    lcap = np.zeros(NG, np.int64)
    col = 0
    a = 0
    while a < NG:
        kind, b = choice[a]
        if kind == "u":
            jl, jh = bucket_wh(a, a + 1)
            lcap[a] = jl
            plan.append(("u", a, jl, jh, col, col + jl * 8))
            col += (jl + jh) * 8
        else:
            jl, jh = bucket_wh(a, b)
            lcap[a:b] = jl
            nb = b - a
            plan.append(("b", a, b, jl, jh, col, col + nb * jl * 8))
            col += nb * (jl + jh) * 8
        a = b if kind == "b" else a + 1
    nix = col
    dmax = int(max(e[-4] + e[-3] for e in plan))

    PAD_L = cfg["PAD_L"]
    PAD_H_REL = cfg["PAD_H"] - HB
    xscale = 127.0 / max(np.abs(x).max(), 1e-6)

    def fill_group(idxc, c, g, jl, jh, ol, oh):
        ml_ = np.full((jl, 128), PAD_L, np.int64)
        mh_ = np.full((jh, 128), PAD_H_REL, np.int64)
        for p in range(128):
            lr = g * 128 + p
            if lr < SHARD:
                s = srcs[c][lr]
                k = min(int(lcap[g]), int(nmf[c, lr]))
                ml_[:k, p] = s[:k]
                mh_[:len(s) - k, p] = s[k:] - HB
        if jl:
            idxc[:, ol:ol + jl * 8] = _wrap_idx(ml_.reshape(-1))
        if jh:
            idxc[:, oh:oh + jh * 8] = _wrap_idx(mh_.reshape(-1))

    per_core = []
    for c in range(R):
        idxc = np.zeros((16, nix), np.int16)
        for e in plan:
            if e[0] == "u":
                _, g, jl, jh, ol, oh = e
                fill_group(idxc, c, g, jl, jh, ol, oh)
            else:
                _, a_, b_, jl, jh, ol, oh = e
                for i, g in enumerate(range(a_, b_)):
                    fill_group(idxc, c, g, jl, jh,
                               ol + i * jl * 8, oh + i * jh * 8)
        nodes_c = byd[c::R]                      # row order
        xm = np.zeros((cfg["IN"], LPAD), np.int8)
        xs = np.asarray(x[nodes_c]).T * xscale
        xm[:, :SHARD] = np.clip(np.round(xs), -127, 127).astype(np.int8)
        per_core.append(dict(idxc=idxc, xmine=xm))
    return per_core, tuple(plan), nix, dmax, xscale, byd


def _aug_w(W, a_s, a_d, nh, hid):
    """[inF, outF+2*nh] = [W.T | As | Ad]."""
    inf = W.shape[1]
    Wr = W.reshape(nh, hid, inf)
    As = np.einsum("hci,hc->ih", Wr, a_s)
    Ad = np.einsum("hci,hc->ih", Wr, a_d)
    return np.concatenate([W.T, As, Ad], axis=1).astype(np.float32)


def build_nc(cfg, plan, nix, dmax):
    R, LPAD, TROWS, SHARD, NG = cfg["R"], cfg["LPAD"], cfg["TROWS"], cfg["SHARD"], cfg["NG"]
    VS, HB = cfg["VSPLIT"], cfg["HBASE"]
    NH, HID, OUT, IN, F = cfg["NH"], cfg["HID"], cfg["OUT"], cfg["IN"], cfg["F"]
    W12, W3 = cfg["W12"], cfg["W3"]             # 264, 66
    GW, GW3 = cfg["GMAX"], cfg["GMAX3"]         # 384, 128
    npad = LPAD - SHARD
    NWELEM = IN * W12 + F * W12 + F * W3        # packed weight f16 elems
    assert NWELEM % R == 0
    WSH = NWELEM // R

    NWELEM += 2 * F + OUT                       # biases ride in the f16 pack
    assert NWELEM % R == 0
    WSH = NWELEM // R

    nc = bacc.Bacc("TRN2", target_bir_lowering=False, debug=False, num_devices=R)

    # single byte-packed input: [wsh f16 | idxc i16 | xmine i8]
    PB_W = WSH * 2
    PB_I = PB_W + 16 * nix * 2
    PBYTES = PB_I + IN * LPAD
    pk = nc.declare_dram_parameter("pk", [1, PBYTES], i8, isOutput=False)
    P = {}
    P["wsh"] = pk[0:1, 0:PB_W].bitcast(f16).rearrange(
        "o (p q) -> (o p) q", q=WSH)
    P["idxc"] = pk[0:1, PB_W:PB_I].bitcast(i16).rearrange(
        "o (p q) -> (o p) q", q=nix)
    P["xmine"] = pk[0:1, PB_I:PBYTES].rearrange("o (p q) -> (o p) q", q=LPAD)
    # packed output: [64 int8 values | f16 scale as 2 bytes]
    out_i8 = nc.declare_dram_parameter("out_i8", [LPAD, OUT + 2], i8, isOutput=True)

    wtmp = nc.dram_tensor("wtmp", [1, WSH], f16)
    wfull = nc.dram_tensor("wfull", [R, WSH], f16, addr_space="Shared")
    # table row stride must be a 256B multiple for dma_gather
    tbl1 = nc.dram_tensor("tbl1", [TROWS, GW], f16, addr_space="Shared")
    tbl2 = nc.dram_tensor("tbl2", [TROWS, GW], f16, addr_space="Shared")
    tbl3 = nc.dram_tensor("tbl3", [TROWS, GW3], f16, addr_space="Shared")
    own_h1 = nc.dram_tensor("own_h1", [LPAD, GW], f16)
    own_h2 = nc.dram_tensor("own_h2", [LPAD, GW], f16)
    own_h3 = nc.dram_tensor("own_h3", [LPAD, GW3], f16)
    own_x1 = nc.dram_tensor("own_x1", [LPAD, F], f16)
    own_x2 = nc.dram_tensor("own_x2", [LPAD, F], f16)

    with tile.TileContext(nc) as tc:
        with tc.tile_pool(name="const", bufs=1) as cpool, \
             tc.tile_pool(name="work", bufs=3) as wpool, \
             tc.tile_pool(name="gath", bufs=2) as gpool, \
             tc.tile_pool(name="adp", bufs=1) as apool, \
             tc.tile_pool(name="pay", bufs=2) as ppool, \
             tc.tile_pool(name="epi", bufs=2) as epool, \
             tc.tile_pool(name="psA", bufs=2, space="PSUM") as psA:

            # ---- weights: shard -> AllGather -> SBUF ----
            nc.sync.dma_start(out=wtmp[:], in_=P["wsh"])
            nc.gpsimd.collective_compute(
                "AllGather", Alu.bypass, replica_groups=[list(range(R))],
                ins=[wtmp[:].opt()], outs=[wfull[:].opt()])
            wflat = wfull[:].rearrange("r q -> (r q)")

            def wload(tag, off, rows, wcols):
                t = cpool.tile([128, wcols], f16, tag=tag)
                nc.sync.dma_start(
                    out=t[0:rows, :],
                    in_=wflat[off:off + rows * wcols].rearrange(
                        "(p q) -> p q", q=wcols))
                return t

            o1 = IN * W12
            o2 = o1 + 128 * W12
            o3 = o2 + 128 * W12
            o4 = o3 + 128 * W3
            w1t = wload("w1t", 0, 128, W12)
            w2lo = wload("w2lo", o1, 128, W12)
            w2hi = wload("w2hi", o2, 128, W12)
            w3lo = wload("w3lo", o3, 128, W3)
            w3hi = wload("w3hi", o4, 128, W3)

            # biases: f16 rows in the weight pack -> broadcast by doubling
            ob1 = o4 + 128 * W3
            ob2 = ob1 + F
            ob3 = ob2 + F

            def bias_bcast(name, off, w):
                t16 = wpool.tile([128, w], f16, tag="b16")
                nc.sync.dma_start(
                    out=t16[0:1, :],
                    in_=wflat[off:off + w].rearrange("(p q) -> p q", q=w))
                p = 1
                while p < 128:
                    nc.sync.dma_start(out=t16[p:2 * p, :], in_=t16[0:p, :])
                    p *= 2
                t = cpool.tile([128, w], f32, tag=name)
                nc.vector.tensor_copy(t[:], t16[:])
                return t

            b1b = bias_bcast("b1", ob1, F)
            b2b = bias_bcast("b2", ob2, F)
            b3b = bias_bcast("b3", ob3, OUT)

            alpha = cpool.tile([128, 1], f32, tag="alpha")
            nc.vector.memset(alpha[:], 0.2)

            padc12 = cpool.tile([128, GW], f16, tag="padc12")
            nc.vector.memset(padc12[:], 0.0)
            nc.vector.memset(padc12[:, F:F + NH], -60000.0)
            padc3 = cpool.tile([128, GW3], f16, tag="padc3")
            nc.vector.memset(padc3[:], 0.0)
            nc.vector.memset(padc3[:, OUT:OUT + 1], -60000.0)

            ixall = cpool.tile([128, nix], i16, tag="ixall")
            for k in range(8):
                nc.sync.dma_start(out=ixall[16 * k:16 * (k + 1), :], in_=P["idxc"])

            # identity idx (row g*128+p) for the own-adst gather, built on
            # device: wrapped layout value at [p, e] = (p % 16) + 16*e
            iw = LPAD // 16
            iop = wpool.tile([128, iw], mybir.dt.int32, tag="iop")
            nc.gpsimd.iota(iop[:], [[0, iw]], channel_multiplier=1)
            nc.vector.tensor_scalar(iop[:], iop[:], 15, None, Alu.bitwise_and)
            ioe = wpool.tile([128, iw], mybir.dt.int32, tag="ioe")
            nc.gpsimd.iota(ioe[:], [[16, iw]], channel_multiplier=0)
            nc.vector.tensor_tensor(iop[:], iop[:], ioe[:], Alu.add)
            ixown = cpool.tile([128, iw], i16, tag="ixown")
            nc.vector.tensor_copy(ixown[:], iop[:])

            def fix_pads(own_h, padc):
                nc.sync.dma_start(out=own_h[SHARD:LPAD, :], in_=padc[:npad, :])

            # ---------------- L1 dense: xmine int8 -> own_h1 ----------------
            with tc.For_i(0, LPAD, 128) as c0:
                xc8 = wpool.tile([IN, 128], i8, tag="xc8")
                nc.sync.dma_start(out=xc8[:], in_=P["xmine"][:, ds(c0, 128)])
                xcf = wpool.tile([IN, 128], f16, tag="xcf")
                nc.vector.tensor_copy(xcf[:], xc8[:])
                ps = psA.tile([128, W12], f32, tag="dens")
                nc.tensor.matmul(ps[:], lhsT=xcf[:], rhs=w1t[:],
                                 start=True, stop=True)
                hrow = wpool.tile([128, W12], f16, tag="hrow")
                nc.scalar.activation(hrow[:], ps[:], Act.Copy)
                nc.sync.dma_start(out=own_h1[ds(c0, 128), 0:W12], in_=hrow[:])

            fix_pads(own_h1, padc12)
            nc.gpsimd.collective_compute(
                "AllGather", Alu.bypass, replica_groups=[list(range(R))],
                ins=[own_h1[:].opt()], outs=[tbl1[:].opt()])

            # ---------------- generic agg layer -----------------------------
            def agg_layer(tbl, own_h, gw, nh, bias_b, sink):
                """gw: table row stride = gather elem_size; sink(rs, acc)
                consumes the [128, nhc+nh] f32 accumulator for the 128 nodes
                whose rows start at `rs` (int or loop scalar)."""
                nhc = nh * HID
                payw = nhc + nh
                # own adst: one identity gather + compact
                adg = apool.tile([128, NG * GW3], f16, tag="adg")
                a3 = adg[:].rearrange("p (g q) -> p g q", q=GW3)
                win = gw - GW3                   # window start: covers adst tail
                nc.gpsimd.dma_gather(
                    a3, own_h[0:LPAD, win:gw], ixown[:], NG * 128, NG * 128,
                    GW3, elem_step=gw, single_packet=False)
                adC = wpool.tile([128, NG * nh], f16, tag=f"adC{nh}")
                aCv = adC[:].rearrange("p (g h) -> p g h", h=nh)
                # adst sits at window cols [nhc+nh-win, nhc+2nh-win)
                a0 = nhc + nh - win
                nc.vector.tensor_copy(aCv[:, :, :], a3[:, :, a0:a0 + nh])

                def group_body(rs, jl, jh, ol, oh, adsl):
                    """rs: node-row start; ol/oh: idx col starts; adsl: adC
                    col start (all ints or loop scalars)."""
                    d = jl + jh
                    gat = gpool.tile([128, dmax * gw], f16, tag="gat")
                    g3 = gat[:].rearrange("p (j q) -> p j q", q=gw)
                    if jl:
                        nc.gpsimd.dma_gather(
                            g3[:, 0:jl, :], tbl[0:VS, :],
                            ixall[:, ds(ol, jl * 8)],
                            jl * 128, jl * 128, gw, single_packet=False)
                    if jh:
                        nc.gpsimd.dma_gather(
                            g3[:, jl:d, :], tbl[HB:TROWS, :],
                            ixall[:, ds(oh, jh * 8)],
                            jh * 128, jh * 128, gw, single_packet=False)
                    gq = gat[:].rearrange("p (j q) -> p q j", q=gw)
                    payT = ppool.tile([128, payw * dmax], f16, tag="payT")
                    pq = payT[:].rearrange("p (q j) -> p q j", j=dmax)
                    eTv = pq[:, nhc:nhc + nh, 0:d]
                    # dynamic-offset APs are only safe on the vector engine;
                    # scalar-engine bias reads ignore the loop offset
                    lgT = wpool.tile([128, nh * dmax], f32, tag="lgT")
                    lgv = lgT[:].rearrange("p (h j) -> p h j", j=dmax)
                    nc.vector.tensor_tensor(
                        lgv[:, :, 0:d], gq[:, nhc:nhc + nh, 0:d],
                        adC[:, ds(adsl, nh)].unsqueeze(2)
                        .to_broadcast([128, nh, d]),
                        Alu.add)
                    lg2 = wpool.tile([128, nh * dmax], f32, tag="lg2m")
                    lg2v = lg2[:].rearrange("p (h j) -> p h j", j=dmax)
                    nc.scalar.activation(lg2v[:, :, 0:d], lgv[:, :, 0:d],
                                         Act.Prelu, alpha=alpha[:])
                    nc.scalar.activation(eTv, lg2v[:, :, 0:d], Act.Exp)
                    # payload h*e
                    pn = payT[:, 0:nhc * dmax].rearrange(
                        "p (h cc j) -> p h cc j", cc=HID, j=dmax)
                    hq = gq[:, 0:nhc, :].rearrange(
                        "p (h cc) j -> p h cc j", cc=HID)
                    nc.vector.tensor_tensor(
                        pn[:, :, :, 0:d], hq[:, :, :, 0:d],
                        eTv.unsqueeze(2).to_broadcast([128, nh, HID, d]),
                        Alu.mult)
                    # fused numerator+denominator reduce
                    acc = epool.tile([128, payw], f32, tag="acc")
                    nc.vector.tensor_reduce(acc[:], pq[:, :, 0:d], AX.X, Alu.add)
                    sink(rs, acc)

                for e in plan:
                    if e[0] == "u":
                        _, g, jl, jh, ol, oh = e
                        group_body(g * 128, jl, jh, ol, oh, g * nh)
                    else:
                        _, a_, b_, jl, jh, ol, oh = e
                        nb = b_ - a_
                        with tc.For_i(0, nb, 1) as i:
                            group_body(i * 128 + a_ * 128, jl, jh,
                                       i * (jl * 8) + ol, i * (jh * 8) + oh,
                                       i * nh + a_ * nh)

            def relu_sink(own_x, nh, bias_b):
                nhc = nh * HID

                def sink(rs, acc):
                    rden = epool.tile([128, nh], f32, tag="rden")
                    nc.vector.reciprocal(rden[:], acc[:, nhc:nhc + nh])
                    ob = epool.tile([128, nhc], f32, tag="ob")
                    obv = ob[:].rearrange("p (h q) -> p h q", q=HID)
                    nc.vector.tensor_tensor(
                        obv, acc[:, 0:nhc].rearrange("p (h q) -> p h q", q=HID),
                        rden[:].unsqueeze(2).to_broadcast([128, nh, HID]),
                        Alu.mult)
                    nc.vector.tensor_tensor(ob[:], ob[:], bias_b[:, 0:nhc],
                                            Alu.add)
                    ob16 = epool.tile([128, nhc], f16, tag="ob16")
                    nc.scalar.activation(ob16[:], ob[:], Act.Relu)
                    nc.sync.dma_start(out=own_x[ds(rs, 128), :], in_=ob16[:])
                return sink

            def quant_sink(rs, acc):
                rden = epool.tile([128, 1], f32, tag="rden3")
                nc.vector.reciprocal(rden[:], acc[:, OUT:OUT + 1])
                ob = epool.tile([128, OUT], f32, tag="ob3")
                nc.vector.tensor_tensor(
                    ob[:], acc[:, 0:OUT], rden[:].to_broadcast([128, OUT]),
                    Alu.mult)
                nc.vector.tensor_tensor(ob[:], ob[:], b3b[:, 0:OUT], Alu.add)
                mx = epool.tile([128, 1], f32, tag="mx")
                nc.vector.tensor_reduce(mx[:], ob[:], AX.X, Alu.max,
                                        apply_absolute_value=True)
                nc.vector.tensor_scalar(mx[:], mx[:], 1e-12, None, Alu.max)
                rmx = epool.tile([128, 1], f32, tag="rmx")
                nc.vector.reciprocal(rmx[:], mx[:])
                nrm = epool.tile([128, OUT], f32, tag="nrm")
                nc.vector.tensor_tensor(
                    nrm[:], ob[:], rmx[:].to_broadcast([128, OUT]), Alu.mult)
                oq = epool.tile([128, OUT + 2], i8, tag="oq")
                nc.vector.tensor_scalar(oq[:, 0:OUT], nrm[:], 127.0, None,
                                        Alu.mult)
                # f16 scale packed into the trailing 2 bytes
                nc.scalar.activation(oq[:, OUT:OUT + 2].bitcast(f16), mx[:],
                                     Act.Copy, scale=1.0 / 127.0)
                nc.sync.dma_start(out=out_i8[ds(rs, 128), :], in_=oq[:])

            # ---------------- dense from own_x via XBAR transpose ------------
            def dense_own(own_x, wlo, whi, own_h, wcols):
                with tc.For_i(0, LPAD, 128) as t0:
                    xT0 = wpool.tile([128, 128], f16, tag="xT0")
                    nc.sync.dma_start_transpose(
                        xT0[:], own_x[ds(t0, 128), 0:128])
                    xT1 = wpool.tile([128, 128], f16, tag="xT1")
                    nc.sync.dma_start_transpose(
                        xT1[:], own_x[ds(t0, 128), 128:256])
                    ps = psA.tile([128, wcols], f32, tag="dens")
                    nc.tensor.matmul(ps[:], lhsT=xT0[:], rhs=wlo[:],
                                     start=True, stop=False)
                    nc.tensor.matmul(ps[:], lhsT=xT1[:], rhs=whi[:],
                                     start=False, stop=True)
                    hrow = wpool.tile([128, wcols], f16, tag="hrow")
                    nc.scalar.activation(hrow[:], ps[:], Act.Copy)
                    nc.sync.dma_start(out=own_h[ds(t0, 128), 0:wcols],
                                      in_=hrow[:])

            # ================= pipeline =================
            agg_layer(tbl1, own_h1, GW, NH, b1b, relu_sink(own_x1, NH, b1b))

            dense_own(own_x1, w2lo, w2hi, own_h2, W12)
            fix_pads(own_h2, padc12)
            nc.gpsimd.collective_compute(
                "AllGather", Alu.bypass, replica_groups=[list(range(R))],
                ins=[own_h2[:].opt()], outs=[tbl2[:].opt()])

            agg_layer(tbl2, own_h2, GW, NH, b2b, relu_sink(own_x2, NH, b2b))

            dense_own(own_x2, w3lo, w3hi, own_h3, W3)
            fix_pads(own_h3, padc3)
            nc.gpsimd.collective_compute(
                "AllGather", Alu.bypass, replica_groups=[list(range(R))],
                ins=[own_h3[:].opt()], outs=[tbl3[:].opt()])

            agg_layer(tbl3, own_h3, GW3, 1, b3b, quant_sink)

    if not nc.is_finalized():
        nc.finalize()
    return nc


def make_inputs(inputs, cfg):
    """Host prep: returns (in_maps, plan, nix, dmax, byd)."""
    x = np.asarray(inputs["x"], np.float32)
    edge_index = np.asarray(inputs["edge_index"])
    NH, HID, OUT, F = cfg["NH"], cfg["HID"], cfg["OUT"], cfg["F"]
    per_core, plan, nix, dmax, xscale, byd = prep_host(x, edge_index, cfg)

    w1t = _aug_w(np.asarray(inputs["W1"], np.float32),
                 np.asarray(inputs["as1"], np.float32),
                 np.asarray(inputs["ad1"], np.float32), NH, HID) / xscale
    w2t = _aug_w(np.asarray(inputs["W2"], np.float32),
                 np.asarray(inputs["as2"], np.float32),
                 np.asarray(inputs["ad2"], np.float32), NH, HID)
    w3t = _aug_w(np.asarray(inputs["W3"], np.float32),
                 np.asarray(inputs["as3"], np.float32),
                 np.asarray(inputs["ad3"], np.float32), 1, OUT)
    wpack = np.concatenate([
        w1t.reshape(-1), w2t.reshape(-1), w3t.reshape(-1),
        np.asarray(inputs["b1"], np.float32).reshape(-1),
        np.asarray(inputs["b2"], np.float32).reshape(-1),
        np.asarray(inputs["b3"], np.float32).reshape(-1),
    ]).astype(np.float16)
    R = cfg["R"]
    assert len(wpack) % R == 0
    WSH = len(wpack) // R

    in_maps = []
    for r in range(R):
        pc = per_core[r]
        buf = (wpack[r * WSH:(r + 1) * WSH].tobytes()
               + pc["idxc"].tobytes() + pc["xmine"].tobytes())
        in_maps.append(dict(pk=np.frombuffer(buf, np.int8).reshape(1, -1)))
    return in_maps, plan, nix, dmax, byd


def assemble_out(res, byd, cfg):
    """Dequantize + un-permute per-core outputs to the global node order."""
    N, R, SHARD = cfg["N"], cfg["R"], cfg["SHARD"]
    OUT = cfg["OUT"]
    out = np.empty((N, OUT), np.float32)
    for c in range(R):
        pk = res.results[c]["out_i8"][:SHARD]
        i8v = pk[:, :OUT].astype(np.float32)
        sc = pk[:, OUT:OUT + 2].copy().view(np.float16).astype(np.float32)
        out[byd[c::R]] = i8v * sc
    return out


_KERNEL_CACHE = {}


def run(inputs, cfg=None, trace=False):
    cfg = cfg or make_cfg()
    in_maps, plan, nix, dmax, byd = make_inputs(inputs, cfg)
    key = (cfg["N"], plan)
    if key not in _KERNEL_CACHE:
        _KERNEL_CACHE[key] = build_nc(cfg, plan, nix, dmax)
    nc = _KERNEL_CACHE[key]
    res = run_bass_kernel_spmd(nc, in_maps, list(range(cfg["R"])), trace=trace)
    return assemble_out(res, byd, cfg), res


def kernel(**inputs):
    out, _ = run(inputs)
    return out


# revision 49
# speedup vs baseline: 1.1189x; 1.1189x over previous
"""3-layer GAT on 8 TRN2 NeuronCores.

Sharding: nodes are globally degree-sorted and dealt round-robin across the
8 cores (rank k -> core k%8, local row k//8), so every core's group g (128
nodes = one SBUF partition-tile) holds nodes of near-identical degree and
the cross-core shared gather plan has minimal slot padding. All per-core
storage is in this local row order, which makes every producer/consumer DMA
contiguous: no indirect scatters, no scatter-row table. The host un-permutes
(and dequantizes) the final output.

Per layer: dense GEMM over own rows -> [h | asrc | adst] rows (f16, 384/128
elem row stride - dma_gather requires 256B-multiple strides) -> AllGather
into a replicated table -> per dst-group one low + one high dma_gather of
the in-edge source rows (int16 indices reach 32768 rows; the two windows
[0,32768) and [TROWS-32768, TROWS) overlap, and the host assigns edges in
the overlap to equalize per-group low/high slot maxima), unnormalized
softmax (exp into the tail of the f16 payload tile, one fused tensor_reduce
over edge slots for numerator+denominator), normalize + bias per group.
Dense layers consume the previous layer's output with dma_start_transpose
(XBAR) loads - no PE transposes. Leaky-relu is one Prelu activation (alpha
AP, vector-engine dynamic slices only - scalar-engine bias APs ignore
For_i-dynamic offsets). The last layer emits int8 values with a per-node
f16 scale packed into trailing bytes; the host dequantizes.

The wall clock is dominated by fixed per-call costs of the axon PJRT path
(fresh-jit lowering scaling with BIR size, ~95ms execute dispatch) and the
~35-45MB/s tunnel, so the design minimizes instruction count (For_i
hardware loops for the dense layers and for degree-bucketed aggregation
groups, DP-optimized unroll/bucket split) and bytes on the wire (one
byte-packed input param: f16 weights+biases | int16 indices | int8 x;
one packed int8+scale output).
"""
import numpy as np

from concourse import bacc, mybir, tile
from concourse.bass import ds
from concourse.bass_utils import run_bass_kernel_spmd

f32 = mybir.dt.float32
f16 = mybir.dt.float16
i8 = mybir.dt.int8
i16 = mybir.dt.int16
Alu = mybir.AluOpType
Act = mybir.ActivationFunctionType
AX = mybir.AxisListType

CFG = dict(
    N=50000, IN=128, HID=64, OUT=64, NH=4, R=8,
    VSPLIT=32768,
)


def make_cfg(**over):
    cfg = dict(CFG)
    cfg.update(over)
    N, R = cfg["N"], cfg["R"]
    assert N % R == 0
    cfg["SHARD"] = N // R                       # 6250
    cfg["LPAD"] = ((cfg["SHARD"] + 127) // 128) * 128   # 6272
    cfg["NG"] = cfg["LPAD"] // 128              # 49
    cfg["TROWS"] = R * cfg["LPAD"]              # 50176
    cfg["F"] = cfg["NH"] * cfg["HID"]           # 256
    cfg["W12"] = cfg["F"] + 2 * cfg["NH"]       # 264 natural row width L1/L2
    cfg["W3"] = cfg["OUT"] + 2                  # 66 natural row width L3
    cfg["GMAX"] = 384                           # gather elem_size L1/L2 (256B mult)
    cfg["GMAX3"] = 128                          # gather elem_size L3
    # low gather window [0, VSPLIT); high window [HBASE, TROWS). The two
    # overlap so the host can balance each node's low/high split.
    cfg["HBASE"] = max(0, cfg["TROWS"] - cfg["VSPLIT"])
    assert cfg["TROWS"] - cfg["HBASE"] <= cfg["VSPLIT"]
    # pad rows (asrc=-60000) for unused gather slots
    cfg["PAD_L"] = cfg["SHARD"]                 # core0 pad row, < VSPLIT
    pad_h = None
    for r in range(R):
        cand = r * cfg["LPAD"] + cfg["SHARD"]
        if cand >= cfg["HBASE"]:
            pad_h = cand
            break
    cfg["PAD_H"] = pad_h
    assert pad_h is not None and pad_h - cfg["HBASE"] < 32768
    return cfg


def _wrap_idx(idx_flat):
    """dma_gather compact int16 index layout: ordinal i at [i%16, i//16]."""
    n = len(idx_flat)
    assert n % 16 == 0
    return np.asarray(idx_flat, np.int16).reshape(n // 16, 16).T


def prep_host(x, edge_index, cfg):
    """Degree-balanced sharding. Returns (per_core dicts, plan, nix, dmax,
    xscale, byd) where plan[g] = (jl, jh, off_l, off_h) shared by all cores
    and byd[k] is the node with global degree rank k."""
    N, R, SHARD, LPAD, NG = cfg["N"], cfg["R"], cfg["SHARD"], cfg["LPAD"], cfg["NG"]
    VS, HB = cfg["VSPLIT"], cfg["HBASE"]
    src = np.concatenate([np.asarray(edge_index[0]), np.arange(N)]).astype(np.int64)
    dst = np.concatenate([np.asarray(edge_index[1]), np.arange(N)]).astype(np.int64)

    deg = np.bincount(dst, minlength=N)
    byd = np.argsort(-deg, kind="stable")       # rank -> node
    rank = np.empty(N, np.int64)
    rank[byd] = np.arange(N)
    core_of = rank % R
    row_of = rank // R                          # local row in [0, SHARD)
    trow_of = core_of * LPAD + row_of           # global table row

    tsrc = trow_of[src]
    dcore = core_of[dst]
    drow = row_of[dst]

    # sort edges by (dst node, src row) so each node's slice is value-sorted:
    # must-low [0,HB) | flexible [HB,VS) | must-high [VS,TROWS)
    node_key_all = dcore * SHARD + drow
    order = np.lexsort((tsrc, node_key_all))
    tsrc_s = tsrc[order]
    node_key = node_key_all[order]
    bounds = np.searchsorted(node_key, np.arange(R * SHARD + 1))

    # per-node (ml, ml+fx) counts; per (core,group) choose the low cap L_g
    # minimizing L + max_high across all cores
    srcs = []            # [core][local row] -> sorted src rows
    nml = np.zeros((R, SHARD), np.int64)
    nmf = np.zeros((R, SHARD), np.int64)
    ndeg = np.zeros((R, SHARD), np.int64)
    for c in range(R):
        per = []
        for lr in range(SHARD):
            s = tsrc_s[bounds[c * SHARD + lr]:bounds[c * SHARD + lr + 1]]
            per.append(s)
            nml[c, lr] = np.searchsorted(s, HB)
            nmf[c, lr] = np.searchsorted(s, VS)
            ndeg[c, lr] = len(s)
        srcs.append(per)

    # Per-group curves for the bucket DP: jh_g(L) = max_n(deg - min(mf, L)),
    # valid for L >= mlmax_g. Bucket [a,b) shares (jl,jh) = argmin_L of
    # (L + max_g jh_g(L)); slot padding trades against For_i loop overhead.
    LMAX = int(nmf.max()) + 1
    Ls = np.arange(LMAX + 1)
    jh_curve = np.zeros((NG, LMAX + 1), np.int64)
    mlmax = np.zeros(NG, np.int64)
    for g in range(NG):
        sl = slice(g * 128, min((g + 1) * 128, SHARD))
        ml = nml[:, sl].reshape(-1)
        mf = nmf[:, sl].reshape(-1)
        dg = ndeg[:, sl].reshape(-1)
        jh_curve[g] = (dg[None, :] - np.minimum(mf[None, :], Ls[:, None])).max(1)
        mlmax[g] = ml.max()

    def bucket_wh(a, b):
        lo = int(mlmax[a:b].max())
        jh = jh_curve[a:b].max(0)
        tot = np.maximum(Ls, lo) + jh
        L = int(np.argmin(tot[lo:]) + lo)
        return int(L), int(jh[L])

    # DP over contiguous splits: unrolled group ~4.1ms; For_i bucket
    # ~23.6ms fixed; each padded slot ~60us of idx H2D.
    UNROLL_US = 4056.0
    BUCKET_US = 23556.0
    SLOT_US = 60.0
    wmin = np.array([sum(bucket_wh(g, g + 1)) for g in range(NG)], np.float64)
    INF = float("inf")
    dp = np.full(NG + 1, INF)
    dp[NG] = 0.0
    choice = [None] * NG
    import os
    force_u = int(os.environ.get("GAT_FORCE_UNROLL", "0"))
    for a in range(NG - 1, -1, -1):
        dp[a] = UNROLL_US + dp[a + 1]
        choice[a] = ("u", a + 1)
        if a < force_u:
            continue
        for b in range(a + 2, NG + 1):
            jl, jh = bucket_wh(a, b)
            pad = ((jl + jh) * (b - a) - wmin[a:b].sum()) * SLOT_US
            c = BUCKET_US + pad + dp[b]
            if c < dp[a]:
                dp[a] = c
                choice[a] = ("b", b)
        # cap inner loop cost: O(NG^2) is fine at NG=49

    # emissions: ("u", g, jl, jh, off_l, off_h) or
    #            ("b", a, b, jl, jh, off_l, off_h)
    plan = []
    lcap = np.zeros(NG, np.int64)
    col = 0
    a = 0
    while a < NG:
        kind, b = choice[a]
        if kind == "u":
            jl, jh = bucket_wh(a, a + 1)
            lcap[a] = jl
            plan.append(("u", a, jl, jh, col, col + jl * 8))
            col += (jl + jh) * 8
        else:
            jl, jh = bucket_wh(a, b)
            lcap[a:b] = jl
            nb = b - a
            plan.append(("b", a, b, jl, jh, col, col + nb * jl * 8))
            col += nb * (jl + jh) * 8
        a = b if kind == "b" else a + 1
    nix = col
    dmax = int(max(e[-4] + e[-3] for e in plan))

    PAD_L = cfg["PAD_L"]
    PAD_H_REL = cfg["PAD_H"] - HB
    xscale = 127.0 / max(np.abs(x).max(), 1e-6)

    def fill_group(idxc, c, g, jl, jh, ol, oh):
        ml_ = np.full((jl, 128), PAD_L, np.int64)
        mh_ = np.full((jh, 128), PAD_H_REL, np.int64)
        for p in range(128):
            lr = g * 128 + p
            if lr < SHARD:
                s = srcs[c][lr]
                k = min(int(lcap[g]), int(nmf[c, lr]))
                ml_[:k, p] = s[:k]
                mh_[:len(s) - k, p] = s[k:] - HB
        if jl:
            idxc[:, ol:ol + jl * 8] = _wrap_idx(ml_.reshape(-1))
        if jh:
            idxc[:, oh:oh + jh * 8] = _wrap_idx(mh_.reshape(-1))

    per_core = []
    for c in range(R):
        idxc = np.zeros((16, nix), np.int16)
        for e in plan:
            if e[0] == "u":
                _, g, jl, jh, ol, oh = e
                fill_group(idxc, c, g, jl, jh, ol, oh)
            else:
                _, a_, b_, jl, jh, ol, oh = e
                for i, g in enumerate(range(a_, b_)):
                    fill_group(idxc, c, g, jl, jh,
                               ol + i * jl * 8, oh + i * jh * 8)
        nodes_c = byd[c::R]                      # row order
        xm = np.zeros((cfg["IN"], LPAD), np.int8)
        xs = np.asarray(x[nodes_c]).T * xscale
        xm[:, :SHARD] = np.clip(np.round(xs), -127, 127).astype(np.int8)
        per_core.append(dict(idxc=idxc, xmine=xm))
    return per_core, tuple(plan), nix, dmax, xscale, byd


def _aug_w(W, a_s, a_d, nh, hid):
    """[inF, outF+2*nh] = [W.T | As | Ad]."""
    inf = W.shape[1]
    Wr = W.reshape(nh, hid, inf)
    As = np.einsum("hci,hc->ih", Wr, a_s)
    Ad = np.einsum("hci,hc->ih", Wr, a_d)
    return np.concatenate([W.T, As, Ad], axis=1).astype(np.float32)


def build_nc(cfg, plan, nix, dmax):
    R, LPAD, TROWS, SHARD, NG = cfg["R"], cfg["LPAD"], cfg["TROWS"], cfg["SHARD"], cfg["NG"]
    VS, HB = cfg["VSPLIT"], cfg["HBASE"]
    NH, HID, OUT, IN, F = cfg["NH"], cfg["HID"], cfg["OUT"], cfg["IN"], cfg["F"]
    W12, W3 = cfg["W12"], cfg["W3"]             # 264, 66
    GW, GW3 = cfg["GMAX"], cfg["GMAX3"]         # 384, 128
    npad = LPAD - SHARD
    NWELEM = IN * W12 + F * W12 + F * W3        # packed weight f16 elems
    assert NWELEM % R == 0
    WSH = NWELEM // R

    NWELEM += 2 * F + OUT                       # biases ride in the f16 pack
    assert NWELEM % R == 0
    WSH = NWELEM // R

    nc = bacc.Bacc("TRN2", target_bir_lowering=False, debug=False, num_devices=R)

    # single byte-packed input: [wsh f16 | idxc i16 | xmine i8]
    PB_W = WSH * 2
    PB_I = PB_W + 16 * nix * 2
    PBYTES = PB_I + IN * LPAD
    pk = nc.declare_dram_parameter("pk", [1, PBYTES], i8, isOutput=False)
    P = {}
    P["wsh"] = pk[0:1, 0:PB_W].bitcast(f16).rearrange(
        "o (p q) -> (o p) q", q=WSH)
    P["idxc"] = pk[0:1, PB_W:PB_I].bitcast(i16).rearrange(
        "o (p q) -> (o p) q", q=nix)
    P["xmine"] = pk[0:1, PB_I:PBYTES].rearrange("o (p q) -> (o p) q", q=LPAD)
    # packed output: [64 int8 values | f16 scale as 2 bytes]
    out_i8 = nc.declare_dram_parameter("out_i8", [LPAD, OUT + 2], i8, isOutput=True)

    wtmp = nc.dram_tensor("wtmp", [1, WSH], f16)
    wfull = nc.dram_tensor("wfull", [R, WSH], f16, addr_space="Shared")
    # table row stride must be a 256B multiple for dma_gather
    tbl1 = nc.dram_tensor("tbl1", [TROWS, GW], f16, addr_space="Shared")
    tbl2 = nc.dram_tensor("tbl2", [TROWS, GW], f16, addr_space="Shared")
    tbl3 = nc.dram_tensor("tbl3", [TROWS, GW3], f16, addr_space="Shared")
    own_h1 = nc.dram_tensor("own_h1", [LPAD, GW], f16)
    own_h2 = nc.dram_tensor("own_h2", [LPAD, GW], f16)
    own_h3 = nc.dram_tensor("own_h3", [LPAD, GW3], f16)
    own_x1 = nc.dram_tensor("own_x1", [LPAD, F], f16)
    own_x2 = nc.dram_tensor("own_x2", [LPAD, F], f16)

    with tile.TileContext(nc) as tc:
        with tc.tile_pool(name="const", bufs=1) as cpool, \
             tc.tile_pool(name="work", bufs=3) as wpool, \
             tc.tile_pool(name="gath", bufs=2) as gpool, \
             tc.tile_pool(name="adp", bufs=1) as apool, \
             tc.tile_pool(name="pay", bufs=2) as ppool, \
             tc.tile_pool(name="epi", bufs=2) as epool, \
             tc.tile_pool(name="psA", bufs=2, space="PSUM") as psA:

            # ---- weights: shard -> AllGather -> SBUF ----
            nc.sync.dma_start(out=wtmp[:], in_=P["wsh"])
            nc.gpsimd.collective_compute(
                "AllGather", Alu.bypass, replica_groups=[list(range(R))],
                ins=[wtmp[:].opt()], outs=[wfull[:].opt()])
            wflat = wfull[:].rearrange("r q -> (r q)")

            def wload(tag, off, rows, wcols):
                t = cpool.tile([128, wcols], f16, tag=tag)
                nc.sync.dma_start(
                    out=t[0:rows, :],
                    in_=wflat[off:off + rows * wcols].rearrange(
                        "(p q) -> p q", q=wcols))
                return t

            o1 = IN * W12
            o2 = o1 + 128 * W12
            o3 = o2 + 128 * W12
            o4 = o3 + 128 * W3
            w1t = wload("w1t", 0, 128, W12)
            w2lo = wload("w2lo", o1, 128, W12)
            w2hi = wload("w2hi", o2, 128, W12)
            w3lo = wload("w3lo", o3, 128, W3)
            w3hi = wload("w3hi", o4, 128, W3)

            # biases: f16 rows in the weight pack -> broadcast by doubling
            ob1 = o4 + 128 * W3
            ob2 = ob1 + F
            ob3 = ob2 + F

            def bias_bcast(name, off, w):
                t16 = wpool.tile([128, w], f16, tag="b16")
                nc.sync.dma_start(
                    out=t16[0:1, :],
                    in_=wflat[off:off + w].rearrange("(p q) -> p q", q=w))
                p = 1
                while p < 128:
                    nc.sync.dma_start(out=t16[p:2 * p, :], in_=t16[0:p, :])
                    p *= 2
                t = cpool.tile([128, w], f32, tag=name)
                nc.vector.tensor_copy(t[:], t16[:])
                return t

            b1b = bias_bcast("b1", ob1, F)
            b2b = bias_bcast("b2", ob2, F)
            b3b = bias_bcast("b3", ob3, OUT)

            alpha = cpool.tile([128, 1], f32, tag="alpha")
            nc.vector.memset(alpha[:], 0.2)

            padc12 = cpool.tile([128, GW], f16, tag="padc12")
            nc.vector.memset(padc12[:], 0.0)
            nc.vector.memset(padc12[:, F:F + NH], -60000.0)
            padc3 = cpool.tile([128, GW3], f16, tag="padc3")
            nc.vector.memset(padc3[:], 0.0)
            nc.vector.memset(padc3[:, OUT:OUT + 1], -60000.0)

            ixall = cpool.tile([128, nix], i16, tag="ixall")
            for k in range(8):
                nc.sync.dma_start(out=ixall[16 * k:16 * (k + 1), :], in_=P["idxc"])

            # identity idx (row g*128+p) for the own-adst gather, built on
            # device: wrapped layout value at [p, e] = (p % 16) + 16*e
            iw = LPAD // 16
            iop = wpool.tile([128, iw], mybir.dt.int32, tag="iop")
            nc.gpsimd.iota(iop[:], [[0, iw]], channel_multiplier=1)
            nc.vector.tensor_scalar(iop[:], iop[:], 15, None, Alu.bitwise_and)
            ioe = wpool.tile([128, iw], mybir.dt.int32, tag="ioe")
            nc.gpsimd.iota(ioe[:], [[16, iw]], channel_multiplier=0)
            nc.vector.tensor_tensor(iop[:], iop[:], ioe[:], Alu.add)
            ixown = cpool.tile([128, iw], i16, tag="ixown")
            nc.vector.tensor_copy(ixown[:], iop[:])

            def fix_pads(own_h, padc):
                nc.sync.dma_start(out=own_h[SHARD:LPAD, :], in_=padc[:npad, :])

            # ---------------- L1 dense: xmine int8 -> own_h1 ----------------
            with tc.For_i(0, LPAD, 128) as c0:
                xc8 = wpool.tile([IN, 128], i8, tag="xc8")
                nc.sync.dma_start(out=xc8[:], in_=P["xmine"][:, ds(c0, 128)])
                xcf = wpool.tile([IN, 128], f16, tag="xcf")
                nc.vector.tensor_copy(xcf[:], xc8[:])
                ps = psA.tile([128, W12], f32, tag="dens")
                nc.tensor.matmul(ps[:], lhsT=xcf[:], rhs=w1t[:],
                                 start=True, stop=True)
                hrow = wpool.tile([128, W12], f16, tag="hrow")
                nc.scalar.activation(hrow[:], ps[:], Act.Copy)
                nc.sync.dma_start(out=own_h1[ds(c0, 128), 0:W12], in_=hrow[:])

            fix_pads(own_h1, padc12)
            nc.gpsimd.collective_compute(
                "AllGather", Alu.bypass, replica_groups=[list(range(R))],
                ins=[own_h1[:].opt()], outs=[tbl1[:].opt()])

            # ---------------- generic agg layer -----------------------------
            def agg_layer(tbl, own_h, gw, nh, bias_b, sink, fuse=None):
                """gw: table row stride = gather elem_size; sink(rs, acc)
                consumes the [128, nhc+nh] f32 accumulator for the 128 nodes
                whose rows start at `rs` (int or loop scalar)."""
                nhc = nh * HID
                payw = nhc + nh
                # own adst: one identity gather + compact
                adg = apool.tile([128, NG * GW3], f16, tag="adg")
                a3 = adg[:].rearrange("p (g q) -> p g q", q=GW3)
                win = gw - GW3                   # window start: covers adst tail
                nc.gpsimd.dma_gather(
                    a3, own_h[0:LPAD, win:gw], ixown[:], NG * 128, NG * 128,
                    GW3, elem_step=gw, single_packet=False)
                adC = wpool.tile([128, NG * nh], f16, tag=f"adC{nh}")
                aCv = adC[:].rearrange("p (g h) -> p g h", h=nh)
                # adst sits at window cols [nhc+nh-win, nhc+2nh-win)
                a0 = nhc + nh - win
                nc.vector.tensor_copy(aCv[:, :, :], a3[:, :, a0:a0 + nh])

                def group_body(rs, jl, jh, ol, oh, adsl):
                    """rs: node-row start; ol/oh: idx col starts; adsl: adC
                    col start (all ints or loop scalars)."""
                    d = jl + jh
                    gat = gpool.tile([128, dmax * gw], f16, tag="gat")
                    g3 = gat[:].rearrange("p (j q) -> p j q", q=gw)
                    if jl:
                        nc.gpsimd.dma_gather(
                            g3[:, 0:jl, :], tbl[0:VS, :],
                            ixall[:, ds(ol, jl * 8)],
                            jl * 128, jl * 128, gw, single_packet=False)
                    if jh:
                        nc.gpsimd.dma_gather(
                            g3[:, jl:d, :], tbl[HB:TROWS, :],
                            ixall[:, ds(oh, jh * 8)],
                            jh * 128, jh * 128, gw, single_packet=False)
                    gq = gat[:].rearrange("p (j q) -> p q j", q=gw)
                    payT = ppool.tile([128, payw * dmax], f16, tag="payT")
                    pq = payT[:].rearrange("p (q j) -> p q j", j=dmax)
                    eTv = pq[:, nhc:nhc + nh, 0:d]
                    # dynamic-offset APs are only safe on the vector engine;
                    # scalar-engine bias reads ignore the loop offset
                    lgT = wpool.tile([128, nh * dmax], f32, tag="lgT")
                    lgv = lgT[:].rearrange("p (h j) -> p h j", j=dmax)
                    nc.vector.tensor_tensor(
                        lgv[:, :, 0:d], gq[:, nhc:nhc + nh, 0:d],
                        adC[:, ds(adsl, nh)].unsqueeze(2)
                        .to_broadcast([128, nh, d]),
                        Alu.add)
                    lg2 = wpool.tile([128, nh * dmax], f32, tag="lg2m")
                    lg2v = lg2[:].rearrange("p (h j) -> p h j", j=dmax)
                    nc.scalar.activation(lg2v[:, :, 0:d], lgv[:, :, 0:d],
                                         Act.Prelu, alpha=alpha[:])
                    nc.scalar.activation(eTv, lg2v[:, :, 0:d], Act.Exp)
                    # payload h*e
                    pn = payT[:, 0:nhc * dmax].rearrange(
                        "p (h cc j) -> p h cc j", cc=HID, j=dmax)
                    hq = gq[:, 0:nhc, :].rearrange(
                        "p (h cc) j -> p h cc j", cc=HID)
                    nc.vector.tensor_tensor(
                        pn[:, :, :, 0:d], hq[:, :, :, 0:d],
                        eTv.unsqueeze(2).to_broadcast([128, nh, HID, d]),
                        Alu.mult)
                    # fused numerator+denominator reduce
                    acc = epool.tile([128, payw], f32, tag="acc")
                    nc.vector.tensor_reduce(acc[:], pq[:, :, 0:d], AX.X, Alu.add)
                    sink(rs, acc)
                    if fuse is not None:
                        fuse(rs)

                for e in plan:
                    if e[0] == "u":
                        _, g, jl, jh, ol, oh = e
                        group_body(g * 128, jl, jh, ol, oh, g * nh)
                    else:
                        _, a_, b_, jl, jh, ol, oh = e
                        nb = b_ - a_
                        with tc.For_i(0, nb, 1) as i:
                            group_body(i * 128 + a_ * 128, jl, jh,
                                       i * (jl * 8) + ol, i * (jh * 8) + oh,
                                       i * nh + a_ * nh)

            def relu_sink(own_x, nh, bias_b):
                nhc = nh * HID

                def sink(rs, acc):
                    rden = epool.tile([128, nh], f32, tag="rden")
                    nc.vector.reciprocal(rden[:], acc[:, nhc:nhc + nh])
                    ob = epool.tile([128, nhc], f32, tag="ob")
                    obv = ob[:].rearrange("p (h q) -> p h q", q=HID)
                    nc.vector.tensor_tensor(
                        obv, acc[:, 0:nhc].rearrange("p (h q) -> p h q", q=HID),
                        rden[:].unsqueeze(2).to_broadcast([128, nh, HID]),
                        Alu.mult)
                    nc.vector.tensor_tensor(ob[:], ob[:], bias_b[:, 0:nhc],
                                            Alu.add)
                    ob16 = epool.tile([128, nhc], f16, tag="ob16")
                    nc.scalar.activation(ob16[:], ob[:], Act.Relu)
                    nc.sync.dma_start(out=own_x[ds(rs, 128), :], in_=ob16[:])
                return sink

            def quant_sink(rs, acc):
                rden = epool.tile([128, 1], f32, tag="rden3")
                nc.vector.reciprocal(rden[:], acc[:, OUT:OUT + 1])
                ob = epool.tile([128, OUT], f32, tag="ob3")
                nc.vector.tensor_tensor(
                    ob[:], acc[:, 0:OUT], rden[:].to_broadcast([128, OUT]),
                    Alu.mult)
                nc.vector.tensor_tensor(ob[:], ob[:], b3b[:, 0:OUT], Alu.add)
                mx = epool.tile([128, 1], f32, tag="mx")
                nc.vector.tensor_reduce(mx[:], ob[:], AX.X, Alu.max,
                                        apply_absolute_value=True)
                nc.vector.tensor_scalar(mx[:], mx[:], 1e-12, None, Alu.max)
                rmx = epool.tile([128, 1], f32, tag="rmx")
                nc.vector.reciprocal(rmx[:], mx[:])
                nrm = epool.tile([128, OUT], f32, tag="nrm")
                nc.vector.tensor_tensor(
                    nrm[:], ob[:], rmx[:].to_broadcast([128, OUT]), Alu.mult)
                oq = epool.tile([128, OUT + 2], i8, tag="oq")
                nc.vector.tensor_scalar(oq[:, 0:OUT], nrm[:], 127.0, None,
                                        Alu.mult)
                # f16 scale packed into the trailing 2 bytes
                nc.scalar.activation(oq[:, OUT:OUT + 2].bitcast(f16), mx[:],
                                     Act.Copy, scale=1.0 / 127.0)
                nc.sync.dma_start(out=out_i8[ds(rs, 128), :], in_=oq[:])

            # ---- dense tile via XBAR transpose, fused into the agg loops ----
            def dense_tile(own_x, wlo, whi, own_h, wcols):
                def fuse(rs):
                    xT0 = wpool.tile([128, 128], f16, tag="xT0")
                    nc.sync.dma_start_transpose(
                        xT0[:], own_x[ds(rs, 128), 0:128])
                    xT1 = wpool.tile([128, 128], f16, tag="xT1")
                    nc.sync.dma_start_transpose(
                        xT1[:], own_x[ds(rs, 128), 128:256])
                    ps = psA.tile([128, wcols], f32, tag="dens")
                    nc.tensor.matmul(ps[:], lhsT=xT0[:], rhs=wlo[:],
                                     start=True, stop=False)
                    nc.tensor.matmul(ps[:], lhsT=xT1[:], rhs=whi[:],
                                     start=False, stop=True)
                    hrow = wpool.tile([128, wcols], f16, tag="hrow")
                    nc.scalar.activation(hrow[:], ps[:], Act.Copy)
                    nc.sync.dma_start(out=own_h[ds(rs, 128), 0:wcols],
                                      in_=hrow[:])
                return fuse

            # ================= pipeline =================
            # dense layer t+1 is fused into agg layer t's group loop: the
            # sink writes own_x rows for a group, the fuse consumes them
            agg_layer(tbl1, own_h1, GW, NH, b1b, relu_sink(own_x1, NH, b1b),
                      fuse=dense_tile(own_x1, w2lo, w2hi, own_h2, W12))
            fix_pads(own_h2, padc12)
            nc.gpsimd.collective_compute(
                "AllGather", Alu.bypass, replica_groups=[list(range(R))],
                ins=[own_h2[:].opt()], outs=[tbl2[:].opt()])

            agg_layer(tbl2, own_h2, GW, NH, b2b, relu_sink(own_x2, NH, b2b),
                      fuse=dense_tile(own_x2, w3lo, w3hi, own_h3, W3))
            fix_pads(own_h3, padc3)
            nc.gpsimd.collective_compute(
                "AllGather", Alu.bypass, replica_groups=[list(range(R))],
                ins=[own_h3[:].opt()], outs=[tbl3[:].opt()])

            agg_layer(tbl3, own_h3, GW3, 1, b3b, quant_sink)

    if not nc.is_finalized():
        nc.finalize()
    return nc


def make_inputs(inputs, cfg):
    """Host prep: returns (in_maps, plan, nix, dmax, byd)."""
    x = np.asarray(inputs["x"], np.float32)
    edge_index = np.asarray(inputs["edge_index"])
    NH, HID, OUT, F = cfg["NH"], cfg["HID"], cfg["OUT"], cfg["F"]
    per_core, plan, nix, dmax, xscale, byd = prep_host(x, edge_index, cfg)

    w1t = _aug_w(np.asarray(inputs["W1"], np.float32),
                 np.asarray(inputs["as1"], np.float32),
                 np.asarray(inputs["ad1"], np.float32), NH, HID) / xscale
    w2t = _aug_w(np.asarray(inputs["W2"], np.float32),
                 np.asarray(inputs["as2"], np.float32),
                 np.asarray(inputs["ad2"], np.float32), NH, HID)
    w3t = _aug_w(np.asarray(inputs["W3"], np.float32),
                 np.asarray(inputs["as3"], np.float32),
                 np.asarray(inputs["ad3"], np.float32), 1, OUT)
    wpack = np.concatenate([
        w1t.reshape(-1), w2t.reshape(-1), w3t.reshape(-1),
        np.asarray(inputs["b1"], np.float32).reshape(-1),
        np.asarray(inputs["b2"], np.float32).reshape(-1),
        np.asarray(inputs["b3"], np.float32).reshape(-1),
    ]).astype(np.float16)
    R = cfg["R"]
    assert len(wpack) % R == 0
    WSH = len(wpack) // R

    in_maps = []
    for r in range(R):
        pc = per_core[r]
        buf = (wpack[r * WSH:(r + 1) * WSH].tobytes()
               + pc["idxc"].tobytes() + pc["xmine"].tobytes())
        in_maps.append(dict(pk=np.frombuffer(buf, np.int8).reshape(1, -1)))
    return in_maps, plan, nix, dmax, byd


def assemble_out(res, byd, cfg):
    """Dequantize + un-permute per-core outputs to the global node order."""
    N, R, SHARD = cfg["N"], cfg["R"], cfg["SHARD"]
    OUT = cfg["OUT"]
    out = np.empty((N, OUT), np.float32)
    for c in range(R):
        pk = res.results[c]["out_i8"][:SHARD]
        i8v = pk[:, :OUT].astype(np.float32)
        sc = pk[:, OUT:OUT + 2].copy().view(np.float16).astype(np.float32)
        out[byd[c::R]] = i8v * sc
    return out


_KERNEL_CACHE = {}


def run(inputs, cfg=None, trace=False):
    cfg = cfg or make_cfg()
    in_maps, plan, nix, dmax, byd = make_inputs(inputs, cfg)
    key = (cfg["N"], plan)
    if key not in _KERNEL_CACHE:
        _KERNEL_CACHE[key] = build_nc(cfg, plan, nix, dmax)
    nc = _KERNEL_CACHE[key]
    res = run_bass_kernel_spmd(nc, in_maps, list(range(cfg["R"])), trace=trace)
    return assemble_out(res, byd, cfg), res


def kernel(**inputs):
    out, _ = run(inputs)
    return out


# revision 51
# speedup vs baseline: 1.1561x; 1.0332x over previous
"""3-layer GAT on 8 TRN2 NeuronCores.

Sharding: nodes are globally degree-sorted and dealt round-robin across the
8 cores (rank k -> core k%8, local row k//8), so every core's group g (128
nodes = one SBUF partition-tile) holds nodes of near-identical degree and
the cross-core shared gather plan has minimal slot padding. All per-core
storage is in this local row order, which makes every producer/consumer DMA
contiguous: no indirect scatters, no scatter-row table. The host un-permutes
(and dequantizes) the final output.

Per layer: dense GEMM over own rows -> [h | asrc | adst] rows (f16, 384/128
elem row stride - dma_gather requires 256B-multiple strides) -> AllGather
into a replicated table -> per dst-group one low + one high dma_gather of
the in-edge source rows (int16 indices reach 32768 rows; the two windows
[0,32768) and [TROWS-32768, TROWS) overlap, and the host assigns edges in
the overlap to equalize per-group low/high slot maxima), unnormalized
softmax (exp into the tail of the f16 payload tile, one fused tensor_reduce
over edge slots for numerator+denominator), normalize + bias per group.
Dense layers consume the previous layer's output with dma_start_transpose
(XBAR) loads - no PE transposes. Leaky-relu is one Prelu activation (alpha
AP, vector-engine dynamic slices only - scalar-engine bias APs ignore
For_i-dynamic offsets). The last layer emits int8 values with a per-node
f16 scale packed into trailing bytes; the host dequantizes.

The wall clock is dominated by fixed per-call costs of the axon PJRT path
(fresh-jit lowering scaling with BIR size, ~95ms execute dispatch) and the
~35-45MB/s tunnel, so the design minimizes instruction count (For_i
hardware loops for the dense layers and for degree-bucketed aggregation
groups, DP-optimized unroll/bucket split) and bytes on the wire (one
byte-packed input param: f16 weights+biases | int16 indices | int8 x;
one packed int8+scale output).
"""
import numpy as np

from concourse import bacc, mybir, tile
from concourse.bass import ds
from concourse.bass_utils import run_bass_kernel_spmd

f32 = mybir.dt.float32
f16 = mybir.dt.float16
i8 = mybir.dt.int8
i16 = mybir.dt.int16
Alu = mybir.AluOpType
Act = mybir.ActivationFunctionType
AX = mybir.AxisListType

CFG = dict(
    N=50000, IN=128, HID=64, OUT=64, NH=4, R=8,
    VSPLIT=32768,
)


def make_cfg(**over):
    cfg = dict(CFG)
    cfg.update(over)
    N, R = cfg["N"], cfg["R"]
    assert N % R == 0
    cfg["SHARD"] = N // R                       # 6250
    cfg["LPAD"] = ((cfg["SHARD"] + 127) // 128) * 128   # 6272
    cfg["NG"] = cfg["LPAD"] // 128              # 49
    cfg["TROWS"] = R * cfg["LPAD"]              # 50176
    cfg["F"] = cfg["NH"] * cfg["HID"]           # 256
    cfg["W12"] = cfg["F"] + 2 * cfg["NH"]       # 264 natural row width L1/L2
    cfg["W3"] = cfg["OUT"] + 2                  # 66 natural row width L3
    cfg["GMAX"] = 384                           # gather elem_size L1/L2 (256B mult)
    cfg["GMAX3"] = 128                          # gather elem_size L3
    # low gather window [0, VSPLIT); high window [HBASE, TROWS). The two
    # overlap so the host can balance each node's low/high split.
    cfg["HBASE"] = max(0, cfg["TROWS"] - cfg["VSPLIT"])
    assert cfg["TROWS"] - cfg["HBASE"] <= cfg["VSPLIT"]
    # pad rows (asrc=-60000) for unused gather slots
    cfg["PAD_L"] = cfg["SHARD"]                 # core0 pad row, < VSPLIT
    pad_h = None
    for r in range(R):
        cand = r * cfg["LPAD"] + cfg["SHARD"]
        if cand >= cfg["HBASE"]:
            pad_h = cand
            break
    cfg["PAD_H"] = pad_h
    assert pad_h is not None and pad_h - cfg["HBASE"] < 32768
    return cfg


def _wrap_idx(idx_flat):
    """dma_gather compact int16 index layout: ordinal i at [i%16, i//16]."""
    n = len(idx_flat)
    assert n % 16 == 0
    return np.asarray(idx_flat, np.int16).reshape(n // 16, 16).T


def prep_host(x, edge_index, cfg):
    """Degree-balanced sharding. Returns (per_core dicts, plan, nix, dmax,
    xscale, byd) where plan[g] = (jl, jh, off_l, off_h) shared by all cores
    and byd[k] is the node with global degree rank k."""
    N, R, SHARD, LPAD, NG = cfg["N"], cfg["R"], cfg["SHARD"], cfg["LPAD"], cfg["NG"]
    VS, HB = cfg["VSPLIT"], cfg["HBASE"]
    src = np.concatenate([np.asarray(edge_index[0]), np.arange(N)]).astype(np.int64)
    dst = np.concatenate([np.asarray(edge_index[1]), np.arange(N)]).astype(np.int64)

    deg = np.bincount(dst, minlength=N)
    byd = np.argsort(-deg, kind="stable")       # rank -> node
    rank = np.empty(N, np.int64)
    rank[byd] = np.arange(N)
    core_of = rank % R
    row_of = rank // R                          # local row in [0, SHARD)
    trow_of = core_of * LPAD + row_of           # global table row

    tsrc = trow_of[src]
    dcore = core_of[dst]
    drow = row_of[dst]

    # sort edges by (dst node, src row) so each node's slice is value-sorted:
    # must-low [0,HB) | flexible [HB,VS) | must-high [VS,TROWS)
    node_key_all = dcore * SHARD + drow
    order = np.lexsort((tsrc, node_key_all))
    tsrc_s = tsrc[order]
    node_key = node_key_all[order]
    bounds = np.searchsorted(node_key, np.arange(R * SHARD + 1))

    # per-node (ml, ml+fx) counts; per (core,group) choose the low cap L_g
    # minimizing L + max_high across all cores
    srcs = []            # [core][local row] -> sorted src rows
    nml = np.zeros((R, SHARD), np.int64)
    nmf = np.zeros((R, SHARD), np.int64)
    ndeg = np.zeros((R, SHARD), np.int64)
    for c in range(R):
        per = []
        for lr in range(SHARD):
            s = tsrc_s[bounds[c * SHARD + lr]:bounds[c * SHARD + lr + 1]]
            per.append(s)
            nml[c, lr] = np.searchsorted(s, HB)
            nmf[c, lr] = np.searchsorted(s, VS)
            ndeg[c, lr] = len(s)
        srcs.append(per)

    # Per-group curves for the bucket DP: jh_g(L) = max_n(deg - min(mf, L)),
    # valid for L >= mlmax_g. Bucket [a,b) shares (jl,jh) = argmin_L of
    # (L + max_g jh_g(L)); slot padding trades against For_i loop overhead.
    LMAX = int(nmf.max()) + 1
    Ls = np.arange(LMAX + 1)
    jh_curve = np.zeros((NG, LMAX + 1), np.int64)
    mlmax = np.zeros(NG, np.int64)
    for g in range(NG):
        sl = slice(g * 128, min((g + 1) * 128, SHARD))
        ml = nml[:, sl].reshape(-1)
        mf = nmf[:, sl].reshape(-1)
        dg = ndeg[:, sl].reshape(-1)
        jh_curve[g] = (dg[None, :] - np.minimum(mf[None, :], Ls[:, None])).max(1)
        mlmax[g] = ml.max()

    def bucket_wh(a, b):
        lo = int(mlmax[a:b].max())
        jh = jh_curve[a:b].max(0)
        tot = np.maximum(Ls, lo) + jh
        L = int(np.argmin(tot[lo:]) + lo)
        return int(L), int(jh[L])

    # DP over contiguous splits: unrolled group ~4.1ms; For_i bucket
    # ~23.6ms fixed; each padded slot ~60us of idx H2D.
    UNROLL_US = 4056.0
    BUCKET_US = 23556.0
    SLOT_US = 60.0
    wmin = np.array([sum(bucket_wh(g, g + 1)) for g in range(NG)], np.float64)
    INF = float("inf")
    dp = np.full(NG + 1, INF)
    dp[NG] = 0.0
    choice = [None] * NG
    import os
    force_u = int(os.environ.get("GAT_FORCE_UNROLL", "0"))
    for a in range(NG - 1, -1, -1):
        dp[a] = UNROLL_US + dp[a + 1]
        choice[a] = ("u", a + 1)
        if a < force_u:
            continue
        for b in range(a + 2, NG + 1):
            jl, jh = bucket_wh(a, b)
            pad = ((jl + jh) * (b - a) - wmin[a:b].sum()) * SLOT_US
            c = BUCKET_US + pad + dp[b]
            if c < dp[a]:
                dp[a] = c
                choice[a] = ("b", b)
        # cap inner loop cost: O(NG^2) is fine at NG=49

    # emissions: ("u", g, jl, jh, off_l, off_h) or
    #            ("b", a, b, jl, jh, off_l, off_h)
    plan = []
    lcap = np.zeros(NG, np.int64)
    col = 0
    a = 0
    while a < NG:
        kind, b = choice[a]
        if kind == "u":
            jl, jh = bucket_wh(a, a + 1)
            lcap[a] = jl
            plan.append(("u", a, jl, jh, col, col + jl * 8))
            col += (jl + jh) * 8
        else:
            jl, jh = bucket_wh(a, b)
            lcap[a:b] = jl
            nb = b - a
            plan.append(("b", a, b, jl, jh, col, col + nb * jl * 8))
            col += nb * (jl + jh) * 8
        a = b if kind == "b" else a + 1
    nix = col
    dmax = int(max(e[-4] + e[-3] for e in plan))

    PAD_L = cfg["PAD_L"]
    PAD_H_REL = cfg["PAD_H"] - HB
    xscale = 127.0 / max(np.abs(x).max(), 1e-6)

    def fill_group(idxc, c, g, jl, jh, ol, oh):
        ml_ = np.full((jl, 128), PAD_L, np.int64)
        mh_ = np.full((jh, 128), PAD_H_REL, np.int64)
        for p in range(128):
            lr = g * 128 + p
            if lr < SHARD:
                s = srcs[c][lr]
                k = min(int(lcap[g]), int(nmf[c, lr]))
                ml_[:k, p] = s[:k]
                mh_[:len(s) - k, p] = s[k:] - HB
        if jl:
            idxc[:, ol:ol + jl * 8] = _wrap_idx(ml_.reshape(-1))
        if jh:
            idxc[:, oh:oh + jh * 8] = _wrap_idx(mh_.reshape(-1))

    per_core = []
    for c in range(R):
        idxc = np.zeros((16, nix), np.int16)
        for e in plan:
            if e[0] == "u":
                _, g, jl, jh, ol, oh = e
                fill_group(idxc, c, g, jl, jh, ol, oh)
            else:
                _, a_, b_, jl, jh, ol, oh = e
                for i, g in enumerate(range(a_, b_)):
                    fill_group(idxc, c, g, jl, jh,
                               ol + i * jl * 8, oh + i * jh * 8)
        nodes_c = byd[c::R]                      # row order
        xm = np.zeros((cfg["IN"], LPAD), np.int8)
        xs = np.asarray(x[nodes_c]).T * xscale
        xm[:, :SHARD] = np.clip(np.round(xs), -127, 127).astype(np.int8)
        per_core.append(dict(idxc=idxc, xmine=xm))
    return per_core, tuple(plan), nix, dmax, xscale, byd


def _aug_w(W, a_s, a_d, nh, hid):
    """[inF, outF+2*nh] = [W.T | As | Ad]."""
    inf = W.shape[1]
    Wr = W.reshape(nh, hid, inf)
    As = np.einsum("hci,hc->ih", Wr, a_s)
    Ad = np.einsum("hci,hc->ih", Wr, a_d)
    return np.concatenate([W.T, As, Ad], axis=1).astype(np.float32)


def build_nc(cfg, plan, nix, dmax):
    R, LPAD, TROWS, SHARD, NG = cfg["R"], cfg["LPAD"], cfg["TROWS"], cfg["SHARD"], cfg["NG"]
    VS, HB = cfg["VSPLIT"], cfg["HBASE"]
    NH, HID, OUT, IN, F = cfg["NH"], cfg["HID"], cfg["OUT"], cfg["IN"], cfg["F"]
    W12, W3 = cfg["W12"], cfg["W3"]             # 264, 66
    GW, GW3 = cfg["GMAX"], cfg["GMAX3"]         # 384, 128
    npad = LPAD - SHARD
    NWELEM = IN * W12 + F * W12 + F * W3        # packed weight f16 elems
    assert NWELEM % R == 0
    WSH = NWELEM // R

    NWELEM += 2 * F + OUT                       # biases ride in the f16 pack
    assert NWELEM % R == 0
    WSH = NWELEM // R

    nc = bacc.Bacc("TRN2", target_bir_lowering=False, debug=False, num_devices=R)

    # single byte-packed input: [wsh f16 | idxc i16 | xmine i8]
    PB_W = WSH * 2
    PB_I = PB_W + 16 * nix * 2
    PBYTES = PB_I + IN * LPAD
    pk = nc.declare_dram_parameter("pk", [1, PBYTES], i8, isOutput=False)
    P = {}
    P["wsh"] = pk[0:1, 0:PB_W].bitcast(f16).rearrange(
        "o (p q) -> (o p) q", q=WSH)
    P["idxc"] = pk[0:1, PB_W:PB_I].bitcast(i16).rearrange(
        "o (p q) -> (o p) q", q=nix)
    P["xmine"] = pk[0:1, PB_I:PBYTES].rearrange("o (p q) -> (o p) q", q=LPAD)
    # packed output: [64 int8 values | f16 scale as 2 bytes]
    out_i8 = nc.declare_dram_parameter("out_i8", [LPAD, OUT + 2], i8, isOutput=True)

    wtmp = nc.dram_tensor("wtmp", [1, WSH], f16)
    wfull = nc.dram_tensor("wfull", [R, WSH], f16, addr_space="Shared")
    # table row stride must be a 256B multiple for dma_gather
    tbl1 = nc.dram_tensor("tbl1", [TROWS, GW], f16, addr_space="Shared")
    tbl2 = nc.dram_tensor("tbl2", [TROWS, GW], f16, addr_space="Shared")
    tbl3 = nc.dram_tensor("tbl3", [TROWS, GW3], f16, addr_space="Shared")
    own_h1 = nc.dram_tensor("own_h1", [LPAD, GW], f16)
    own_h2 = nc.dram_tensor("own_h2", [LPAD, GW], f16)
    own_h3 = nc.dram_tensor("own_h3", [LPAD, GW3], f16)
    own_x1 = nc.dram_tensor("own_x1", [LPAD, F], f16)
    own_x2 = nc.dram_tensor("own_x2", [LPAD, F], f16)

    with tile.TileContext(nc) as tc:
        with tc.tile_pool(name="const", bufs=1) as cpool, \
             tc.tile_pool(name="work", bufs=3) as wpool, \
             tc.tile_pool(name="gath", bufs=2) as gpool, \
             tc.tile_pool(name="adp", bufs=1) as apool, \
             tc.tile_pool(name="pay", bufs=2) as ppool, \
             tc.tile_pool(name="epi", bufs=2) as epool, \
             tc.tile_pool(name="psA", bufs=2, space="PSUM") as psA:

            # ---- weights: shard -> AllGather -> SBUF ----
            nc.sync.dma_start(out=wtmp[:], in_=P["wsh"])
            nc.gpsimd.collective_compute(
                "AllGather", Alu.bypass, replica_groups=[list(range(R))],
                ins=[wtmp[:].opt()], outs=[wfull[:].opt()])
            wflat = wfull[:].rearrange("r q -> (r q)")

            def wload(tag, off, rows, wcols):
                t = cpool.tile([128, wcols], f16, tag=tag)
                nc.sync.dma_start(
                    out=t[0:rows, :],
                    in_=wflat[off:off + rows * wcols].rearrange(
                        "(p q) -> p q", q=wcols))
                return t

            o1 = IN * W12
            o2 = o1 + 128 * W12
            o3 = o2 + 128 * W12
            o4 = o3 + 128 * W3
            w1t = wload("w1t", 0, 128, W12)
            w2lo = wload("w2lo", o1, 128, W12)
            w2hi = wload("w2hi", o2, 128, W12)
            w3lo = wload("w3lo", o3, 128, W3)
            w3hi = wload("w3hi", o4, 128, W3)

            # biases: f16 rows in the weight pack -> broadcast by doubling
            ob1 = o4 + 128 * W3
            ob2 = ob1 + F
            ob3 = ob2 + F

            # one doubling chain broadcasts all three contiguous bias rows
            BW = 2 * F + OUT
            bt16 = wpool.tile([128, BW], f16, tag="b16")
            nc.sync.dma_start(
                out=bt16[0:1, :],
                in_=wflat[ob1:ob1 + BW].rearrange("(p q) -> p q", q=BW))
            p = 1
            while p < 128:
                nc.sync.dma_start(out=bt16[p:2 * p, :], in_=bt16[0:p, :])
                p *= 2
            ballb = cpool.tile([128, BW], f32, tag="ballb")
            nc.vector.tensor_copy(ballb[:], bt16[:])
            b1b = ballb[:, 0:F]
            b2b = ballb[:, F:2 * F]
            b3b = ballb[:, 2 * F:2 * F + OUT]

            alpha = cpool.tile([128, 1], f32, tag="alpha")
            nc.vector.memset(alpha[:], 0.2)

            padc12 = cpool.tile([128, GW], f16, tag="padc12")
            nc.vector.memset(padc12[:], 0.0)
            nc.vector.memset(padc12[:, F:F + NH], -60000.0)
            padc3 = cpool.tile([128, GW3], f16, tag="padc3")
            nc.vector.memset(padc3[:], 0.0)
            nc.vector.memset(padc3[:, OUT:OUT + 1], -60000.0)

            ixall = cpool.tile([128, nix], i16, tag="ixall")
            nc.sync.dma_start(out=ixall[0:16, :], in_=P["idxc"])
            p = 16
            while p < 128:
                nc.sync.dma_start(out=ixall[p:2 * p, :], in_=ixall[0:p, :])
                p *= 2

            # identity idx (row g*128+p) for the own-adst gather, built on
            # device: wrapped layout value at [p, e] = (p % 16) + 16*e
            iw = LPAD // 16
            iop = wpool.tile([128, iw], mybir.dt.int32, tag="iop")
            nc.gpsimd.iota(iop[:], [[0, iw]], channel_multiplier=1)
            nc.vector.tensor_scalar(iop[:], iop[:], 15, None, Alu.bitwise_and)
            ioe = wpool.tile([128, iw], mybir.dt.int32, tag="ioe")
            nc.gpsimd.iota(ioe[:], [[16, iw]], channel_multiplier=0)
            nc.vector.tensor_tensor(iop[:], iop[:], ioe[:], Alu.add)
            ixown = cpool.tile([128, iw], i16, tag="ixown")
            nc.vector.tensor_copy(ixown[:], iop[:])

            def fix_pads(own_h, padc):
                nc.sync.dma_start(out=own_h[SHARD:LPAD, :], in_=padc[:npad, :])

            # ---------------- L1 dense: xmine int8 -> own_h1 ----------------
            with tc.For_i(0, LPAD, 128) as c0:
                xc8 = wpool.tile([IN, 128], i8, tag="xc8")
                nc.sync.dma_start(out=xc8[:], in_=P["xmine"][:, ds(c0, 128)])
                xcf = wpool.tile([IN, 128], f16, tag="xcf")
                nc.vector.tensor_copy(xcf[:], xc8[:])
                ps = psA.tile([128, W12], f32, tag="dens")
                nc.tensor.matmul(ps[:], lhsT=xcf[:], rhs=w1t[:],
                                 start=True, stop=True)
                hrow = wpool.tile([128, W12], f16, tag="hrow")
                nc.scalar.activation(hrow[:], ps[:], Act.Copy)
                nc.sync.dma_start(out=own_h1[ds(c0, 128), 0:W12], in_=hrow[:])

            fix_pads(own_h1, padc12)
            nc.gpsimd.collective_compute(
                "AllGather", Alu.bypass, replica_groups=[list(range(R))],
                ins=[own_h1[:].opt()], outs=[tbl1[:].opt()])

            # ---------------- generic agg layer -----------------------------
            def agg_layer(tbl, own_h, gw, nh, bias_b, sink, fuse=None):
                """gw: table row stride = gather elem_size; sink(rs, acc)
                consumes the [128, nhc+nh] f32 accumulator for the 128 nodes
                whose rows start at `rs` (int or loop scalar)."""
                nhc = nh * HID
                payw = nhc + nh
                # own adst: one identity gather + compact
                adg = apool.tile([128, NG * GW3], f16, tag="adg")
                a3 = adg[:].rearrange("p (g q) -> p g q", q=GW3)
                win = gw - GW3                   # window start: covers adst tail
                nc.gpsimd.dma_gather(
                    a3, own_h[0:LPAD, win:gw], ixown[:], NG * 128, NG * 128,
                    GW3, elem_step=gw, single_packet=False)
                adC = wpool.tile([128, NG * nh], f16, tag=f"adC{nh}")
                aCv = adC[:].rearrange("p (g h) -> p g h", h=nh)
                # adst sits at window cols [nhc+nh-win, nhc+2nh-win)
                a0 = nhc + nh - win
                nc.vector.tensor_copy(aCv[:, :, :], a3[:, :, a0:a0 + nh])

                def group_body(rs, jl, jh, ol, oh, adsl):
                    """rs: node-row start; ol/oh: idx col starts; adsl: adC
                    col start (all ints or loop scalars)."""
                    d = jl + jh
                    gat = gpool.tile([128, dmax * gw], f16, tag="gat")
                    g3 = gat[:].rearrange("p (j q) -> p j q", q=gw)
                    if jl:
                        nc.gpsimd.dma_gather(
                            g3[:, 0:jl, :], tbl[0:VS, :],
                            ixall[:, ds(ol, jl * 8)],
                            jl * 128, jl * 128, gw, single_packet=False)
                    if jh:
                        nc.gpsimd.dma_gather(
                            g3[:, jl:d, :], tbl[HB:TROWS, :],
                            ixall[:, ds(oh, jh * 8)],
                            jh * 128, jh * 128, gw, single_packet=False)
                    gq = gat[:].rearrange("p (j q) -> p q j", q=gw)
                    payT = ppool.tile([128, payw * dmax], f16, tag="payT")
                    pq = payT[:].rearrange("p (q j) -> p q j", j=dmax)
                    eTv = pq[:, nhc:nhc + nh, 0:d]
                    # dynamic-offset APs are only safe on the vector engine;
                    # scalar-engine bias reads ignore the loop offset
                    lgT = wpool.tile([128, nh * dmax], f32, tag="lgT")
                    lgv = lgT[:].rearrange("p (h j) -> p h j", j=dmax)
                    nc.vector.tensor_tensor(
                        lgv[:, :, 0:d], gq[:, nhc:nhc + nh, 0:d],
                        adC[:, ds(adsl, nh)].unsqueeze(2)
                        .to_broadcast([128, nh, d]),
                        Alu.add)
                    lg2 = wpool.tile([128, nh * dmax], f32, tag="lg2m")
                    lg2v = lg2[:].rearrange("p (h j) -> p h j", j=dmax)
                    nc.scalar.activation(lg2v[:, :, 0:d], lgv[:, :, 0:d],
                                         Act.Prelu, alpha=alpha[:])
                    nc.scalar.activation(eTv, lg2v[:, :, 0:d], Act.Exp)
                    # payload h*e
                    pn = payT[:, 0:nhc * dmax].rearrange(
                        "p (h cc j) -> p h cc j", cc=HID, j=dmax)
                    hq = gq[:, 0:nhc, :].rearrange(
                        "p (h cc) j -> p h cc j", cc=HID)
                    nc.vector.tensor_tensor(
                        pn[:, :, :, 0:d], hq[:, :, :, 0:d],
                        eTv.unsqueeze(2).to_broadcast([128, nh, HID, d]),
                        Alu.mult)
                    # fused numerator+denominator reduce
                    acc = epool.tile([128, payw], f32, tag="acc")
                    nc.vector.tensor_reduce(acc[:], pq[:, :, 0:d], AX.X, Alu.add)
                    sink(rs, acc)
                    if fuse is not None:
                        fuse(rs)

                for e in plan:
                    if e[0] == "u":
                        _, g, jl, jh, ol, oh = e
                        group_body(g * 128, jl, jh, ol, oh, g * nh)
                    else:
                        _, a_, b_, jl, jh, ol, oh = e
                        nb = b_ - a_
                        with tc.For_i(0, nb, 1) as i:
                            group_body(i * 128 + a_ * 128, jl, jh,
                                       i * (jl * 8) + ol, i * (jh * 8) + oh,
                                       i * nh + a_ * nh)

            def relu_sink(own_x, nh, bias_b):
                nhc = nh * HID

                def sink(rs, acc):
                    rden = epool.tile([128, nh], f32, tag="rden")
                    nc.vector.reciprocal(rden[:], acc[:, nhc:nhc + nh])
                    ob = epool.tile([128, nhc], f32, tag="ob")
                    obv = ob[:].rearrange("p (h q) -> p h q", q=HID)
                    nc.vector.tensor_tensor(
                        obv, acc[:, 0:nhc].rearrange("p (h q) -> p h q", q=HID),
                        rden[:].unsqueeze(2).to_broadcast([128, nh, HID]),
                        Alu.mult)
                    nc.vector.tensor_tensor(ob[:], ob[:], bias_b[:, 0:nhc],
                                            Alu.add)
                    ob16 = epool.tile([128, nhc], f16, tag="ob16")
                    nc.scalar.activation(ob16[:], ob[:], Act.Relu)
                    nc.sync.dma_start(out=own_x[ds(rs, 128), :], in_=ob16[:])
                return sink

            def quant_sink(rs, acc):
                rden = epool.tile([128, 1], f32, tag="rden3")
                nc.vector.reciprocal(rden[:], acc[:, OUT:OUT + 1])
                ob = epool.tile([128, OUT], f32, tag="ob3")
                nc.vector.tensor_tensor(
                    ob[:], acc[:, 0:OUT], rden[:].to_broadcast([128, OUT]),
                    Alu.mult)
                nc.vector.tensor_tensor(ob[:], ob[:], b3b[:, 0:OUT], Alu.add)
                mx = epool.tile([128, 1], f32, tag="mx")
                nc.vector.tensor_reduce(mx[:], ob[:], AX.X, Alu.max,
                                        apply_absolute_value=True)
                nc.vector.tensor_scalar(mx[:], mx[:], 1e-12, None, Alu.max)
                rmx = epool.tile([128, 1], f32, tag="rmx")
                nc.vector.reciprocal(rmx[:], mx[:])
                nrm = epool.tile([128, OUT], f32, tag="nrm")
                nc.vector.tensor_tensor(
                    nrm[:], ob[:], rmx[:].to_broadcast([128, OUT]), Alu.mult)
                oq = epool.tile([128, OUT + 2], i8, tag="oq")
                nc.vector.tensor_scalar(oq[:, 0:OUT], nrm[:], 127.0, None,
                                        Alu.mult)
                # f16 scale packed into the trailing 2 bytes
                nc.scalar.activation(oq[:, OUT:OUT + 2].bitcast(f16), mx[:],
                                     Act.Copy, scale=1.0 / 127.0)
                nc.sync.dma_start(out=out_i8[ds(rs, 128), :], in_=oq[:])

            # ---- dense tile via XBAR transpose, fused into the agg loops ----
            def dense_tile(own_x, wlo, whi, own_h, wcols):
                def fuse(rs):
                    xT0 = wpool.tile([128, 128], f16, tag="xT0")
                    nc.sync.dma_start_transpose(
                        xT0[:], own_x[ds(rs, 128), 0:128])
                    xT1 = wpool.tile([128, 128], f16, tag="xT1")
                    nc.sync.dma_start_transpose(
                        xT1[:], own_x[ds(rs, 128), 128:256])
                    ps = psA.tile([128, wcols], f32, tag="dens")
                    nc.tensor.matmul(ps[:], lhsT=xT0[:], rhs=wlo[:],
                                     start=True, stop=False)
                    nc.tensor.matmul(ps[:], lhsT=xT1[:], rhs=whi[:],
                                     start=False, stop=True)
                    hrow = wpool.tile([128, wcols], f16, tag="hrow")
                    nc.scalar.activation(hrow[:], ps[:], Act.Copy)
                    nc.sync.dma_start(out=own_h[ds(rs, 128), 0:wcols],
                                      in_=hrow[:])
                return fuse

            # ================= pipeline =================
            # dense layer t+1 is fused into agg layer t's group loop: the
            # sink writes own_x rows for a group, the fuse consumes them
            agg_layer(tbl1, own_h1, GW, NH, b1b, relu_sink(own_x1, NH, b1b),
                      fuse=dense_tile(own_x1, w2lo, w2hi, own_h2, W12))
            fix_pads(own_h2, padc12)
            nc.gpsimd.collective_compute(
                "AllGather", Alu.bypass, replica_groups=[list(range(R))],
                ins=[own_h2[:].opt()], outs=[tbl2[:].opt()])

            agg_layer(tbl2, own_h2, GW, NH, b2b, relu_sink(own_x2, NH, b2b),
                      fuse=dense_tile(own_x2, w3lo, w3hi, own_h3, W3))
            fix_pads(own_h3, padc3)
            nc.gpsimd.collective_compute(
                "AllGather", Alu.bypass, replica_groups=[list(range(R))],
                ins=[own_h3[:].opt()], outs=[tbl3[:].opt()])

            agg_layer(tbl3, own_h3, GW3, 1, b3b, quant_sink)

    if not nc.is_finalized():
        nc.finalize()
    return nc


def make_inputs(inputs, cfg):
    """Host prep: returns (in_maps, plan, nix, dmax, byd)."""
    x = np.asarray(inputs["x"], np.float32)
    edge_index = np.asarray(inputs["edge_index"])
    NH, HID, OUT, F = cfg["NH"], cfg["HID"], cfg["OUT"], cfg["F"]
    per_core, plan, nix, dmax, xscale, byd = prep_host(x, edge_index, cfg)

    w1t = _aug_w(np.asarray(inputs["W1"], np.float32),
                 np.asarray(inputs["as1"], np.float32),
                 np.asarray(inputs["ad1"], np.float32), NH, HID) / xscale
    w2t = _aug_w(np.asarray(inputs["W2"], np.float32),
                 np.asarray(inputs["as2"], np.float32),
                 np.asarray(inputs["ad2"], np.float32), NH, HID)
    w3t = _aug_w(np.asarray(inputs["W3"], np.float32),
                 np.asarray(inputs["as3"], np.float32),
                 np.asarray(inputs["ad3"], np.float32), 1, OUT)
    wpack = np.concatenate([
        w1t.reshape(-1), w2t.reshape(-1), w3t.reshape(-1),
        np.asarray(inputs["b1"], np.float32).reshape(-1),
        np.asarray(inputs["b2"], np.float32).reshape(-1),
        np.asarray(inputs["b3"], np.float32).reshape(-1),
    ]).astype(np.float16)
    R = cfg["R"]
    assert len(wpack) % R == 0
    WSH = len(wpack) // R

    in_maps = []
    for r in range(R):
        pc = per_core[r]
        buf = (wpack[r * WSH:(r + 1) * WSH].tobytes()
               + pc["idxc"].tobytes() + pc["xmine"].tobytes())
        in_maps.append(dict(pk=np.frombuffer(buf, np.int8).reshape(1, -1)))
    return in_maps, plan, nix, dmax, byd


def assemble_out(res, byd, cfg):
    """Dequantize + un-permute per-core outputs to the global node order."""
    N, R, SHARD = cfg["N"], cfg["R"], cfg["SHARD"]
    OUT = cfg["OUT"]
    out = np.empty((N, OUT), np.float32)
    for c in range(R):
        pk = res.results[c]["out_i8"][:SHARD]
        i8v = pk[:, :OUT].astype(np.float32)
        sc = pk[:, OUT:OUT + 2].copy().view(np.float16).astype(np.float32)
        out[byd[c::R]] = i8v * sc
    return out


_KERNEL_CACHE = {}


def run(inputs, cfg=None, trace=False):
    cfg = cfg or make_cfg()
    in_maps, plan, nix, dmax, byd = make_inputs(inputs, cfg)
    key = (cfg["N"], plan)
    if key not in _KERNEL_CACHE:
        _KERNEL_CACHE[key] = build_nc(cfg, plan, nix, dmax)
    nc = _KERNEL_CACHE[key]
    res = run_bass_kernel_spmd(nc, in_maps, list(range(cfg["R"])), trace=trace)
    return assemble_out(res, byd, cfg), res


def kernel(**inputs):
    out, _ = run(inputs)
    return out
